# revision 1
# baseline (speedup 1.0000x reference)
"""AGNN (2-layer) distributed Bass kernel for 8 TRN2 NeuronCores.

Design:
- Nodes degree-sorted and dealt round-robin to 8 cores (12544 padded rows each,
  98 tiles of 128). All index remapping done on host; output un-permuted on host.
- Per AGNN layer: each core builds a bf16 "table" shard (rows: xn[32] bf16,
  norm f32, zero pad to 256B), AllGather -> full table in DRAM.
- Messages: dst-major slot grid [128 nodes, K slots] per tile, slots bucketed by
  src bank (4 banks of 25088 rows so dma_gather's int16 indices fit). Bulk
  row-major gathers via dma_gather (256B rows); pad slots point at a zero row.
- Compute: DVE mul + grouped reduce for cos logits, ACT exp with fused
  denominator (accum_out), Sigma-e minus host-precomputed pad count, weighted sum
  via PE identity-matmul with PSUM accumulation, ACT relu+scale epilogue.
- lin1 uses host-transposed features; lin2 + log_softmax fused per tile.
"""

import numpy as np

N_NODES = 100000
N_EDGES = 1600000
IN_SIZE = 256
HID = 32
OUT_SIZE = 64
EPS = 1e-12

NCORES = 8
TILES = 98
SHARD = TILES * 128            # 12544
PAD_NODES = NCORES * SHARD     # 100352
NBANKS = 4
BROWS = 2 * SHARD              # 25088 rows per bank (2 shards)
RW = 128                       # bf16 elems per table row = 256B
DUMMY_LOCAL = 12500            # zero row within the first shard of each bank
TG = 2                         # tiles per gather group
NGROUPS = TILES // TG          # 49
PSUM_ACC_TRICK = False          # broadcast-PSUM-out matmul k-reduction

_cache = {}


def _host_preprocess(edge):
    src = np.asarray(edge[0], dtype=np.int64)
    dst = np.asarray(edge[1], dtype=np.int64)
    deg = np.bincount(dst, minlength=N_NODES)
    order = np.argsort(-deg, kind="stable")      # node ids, heavy first
    rank = np.empty(N_NODES, dtype=np.int64)
    rank[order] = np.arange(N_NODES)
    core_of = rank % NCORES
    pos_of = rank // NCORES                      # 0..12499
    grow_of = core_of * SHARD + pos_of           # global padded table row

    # Pass 2: re-sort nodes WITHIN each shard by per-bank src-count vector.
    # Within-shard reordering never changes any node's bank (banks = 2 whole
    # shards), so bank counts computed from the pass-1 layout stay valid.
    bank1 = grow_of[src] // BROWS
    cnt = np.zeros((N_NODES, NBANKS), dtype=np.int32)
    np.add.at(cnt, (dst, bank1), 1)
    for c in range(NCORES):
        nodes_c = np.where(core_of == c)[0]
        key = np.lexsort((-cnt[nodes_c, 3], -cnt[nodes_c, 2],
                          -cnt[nodes_c, 1], -cnt[nodes_c, 0]))
        pos_of[nodes_c[key]] = np.arange(len(nodes_c))
    grow_of = core_of * SHARD + pos_of

    e_core = core_of[dst]
    e_tile = pos_of[dst] // 128
    e_p = pos_of[dst] % 128
    e_srow = grow_of[src]
    e_bank = e_srow // BROWS
    e_local = e_srow % BROWS

    # counts per (core, tile, p, bank)
    key = ((e_core * TILES + e_tile) * 128 + e_p) * NBANKS + e_bank
    counts = np.bincount(key, minlength=NCORES * TILES * 128 * NBANKS)
    counts = counts.reshape(NCORES, TILES, 128, NBANKS)
    KHAT = counts.max(axis=(0, 2))               # [TILES, NBANKS]
    # k-rank of each edge within its (core,tile,p,bank) cell
    sort_idx = np.argsort(key, kind="stable")
    ks = key[sort_idx]
    first = np.r_[True, ks[1:] != ks[:-1]]
    grp_start = np.maximum.accumulate(np.where(first, np.arange(len(ks)), 0))
    e_k = np.empty(len(ks), dtype=np.int64)
    e_k[sort_idx] = np.arange(len(ks)) - grp_start

    # slot grids: per (core, tile, bank): [KHAT[t,b], 128] int16 local idx
    koff = np.zeros((TILES, NBANKS), dtype=np.int64)   # k-offset of (t,b) within tile's concat
    run = np.cumsum(KHAT, axis=1)
    koff[:, 1:] = run[:, :-1]
    KSUM_T = KHAT.sum(axis=1)                          # slots-k per tile
    tile_off = np.r_[0, np.cumsum(KSUM_T)][:-1]        # k-offset of tile within core stream
    TOTK = int(KSUM_T.sum())

    grid = np.full((NCORES, TOTK, 128), DUMMY_LOCAL, dtype=np.int16)
    flat_k = tile_off[e_tile] + koff[e_tile, e_bank] + e_k
    grid[e_core, flat_k, e_p] = e_local.astype(np.int16)

    # per-(group,bank) gather streams: order = for t in group: k-major, p inner
    # stream for (g,b) = concat over t in g of grid[:, tile_off[t]+koff[t,b] : +KHAT[t,b], :]
    # global idx blob per core: concat streams over (g, b), wrapped 16 and replicated x8
    blobs = []
    call_meta = []   # (g, b, off_in_blob_cols, num_idxs, [KHAT[t,b] for t in g])
    col_off = 0
    for g in range(NGROUPS):
        ts = range(g * TG, (g + 1) * TG)
        for b in range(NBANKS):
            parts = [grid[:, tile_off[t] + koff[t, b]: tile_off[t] + koff[t, b] + KHAT[t, b], :]
                     for t in ts]
            st = np.concatenate(parts, axis=1)          # [NCORES, sumK, 128]
            n_idx = st.shape[1] * 128
            stream = st.reshape(NCORES, -1)             # slot j = k*128+p order
            w = n_idx // 16
            wrapped = stream.reshape(NCORES, w, 16).transpose(0, 2, 1)  # [NCORES,16,w]
            blobs.append(np.tile(wrapped, (1, 8, 1)))   # [NCORES, 128, w]
            call_meta.append((g, b, col_off, n_idx, [int(KHAT[t, b]) for t in ts]))
            col_off += w
    idx_blob = np.concatenate(blobs, axis=2)            # [NCORES, 128, WTOT]

    npad = (KSUM_T[:, None].repeat(128, axis=1)[None] * 1.0
            - counts.sum(axis=3).astype(np.float64))    # [NCORES? broadcast] -> shape fix
    npad = (np.broadcast_to(KSUM_T[None, :, None], (NCORES, TILES, 128)).astype(np.float64)
            - counts.sum(axis=3))
    npad = npad.reshape(NCORES, SHARD, 1).astype(np.float32)

    meta = {
        "KHAT": KHAT, "koff": koff, "tile_off": tile_off, "TOTK": TOTK,
        "call_meta": call_meta, "WTOT": int(idx_blob.shape[2]),
        "order": order, "core_of": core_of, "pos_of": pos_of,
    }
    return idx_blob, npad, meta


def _build_program(meta):
    import concourse.bass as bass
    import concourse.bacc as bacc
    import concourse.mybir as mybir
    import concourse.tile as tile
    from concourse.masks import make_identity

    f32 = mybir.dt.float32
    bf16 = mybir.dt.bfloat16
    AF = mybir.ActivationFunctionType
    ALU = mybir.AluOpType

    KHAT = meta["KHAT"]; koff = meta["koff"]; tile_off = meta["tile_off"]
    call_meta = meta["call_meta"]; WTOT = meta["WTOT"]
    KSUM_T = KHAT.sum(axis=1)
    KB_MAX = [int(max(sum(KHAT[t, b] for t in range(g * TG, (g + 1) * TG))
                      for g in range(NGROUPS))) for b in range(NBANKS)]
    KS_MAX = int(max(KSUM_T))

    nc = bacc.Bacc("TRN2", target_bir_lowering=False, debug=False,
                   enable_asserts=False, num_devices=NCORES)
    featsT = nc.dram_tensor("featsT", [IN_SIZE, SHARD], f32, kind="ExternalInput")
    W1_in = nc.dram_tensor("W1", [IN_SIZE, HID], f32, kind="ExternalInput")
    b1_in = nc.dram_tensor("b1", [1, HID], f32, kind="ExternalInput")
    W2_in = nc.dram_tensor("W2", [HID, OUT_SIZE], f32, kind="ExternalInput")
    b2_in = nc.dram_tensor("b2", [1, OUT_SIZE], f32, kind="ExternalInput")
    betas_in = nc.dram_tensor("betas", [1, 2], f32, kind="ExternalInput")
    idx_in = nc.dram_tensor("idx", [128, WTOT], mybir.dt.int16, kind="ExternalInput")
    npad_in = nc.dram_tensor("npad", [SHARD, 1], f32, kind="ExternalInput")
    out_t = nc.dram_tensor("out", [SHARD, OUT_SIZE], f32, kind="ExternalOutput")

    tab_in = [nc.dram_tensor(f"tabin{l}", [SHARD, RW], bf16, kind="Internal")
              for l in range(2)]
    tab_out = [nc.dram_tensor(f"tabout{l}", [PAD_NODES, RW], bf16, kind="Internal",
                              addr_space="Shared") for l in range(2)]

    with tile.TileContext(nc) as tc:
        with tc.tile_pool(name="const", bufs=1) as cpool, \
             tc.tile_pool(name="work", bufs=2) as pool, \
             tc.tile_pool(name="slab", bufs=2) as spool, \
             tc.tile_pool(name="psum", bufs=1, space="PSUM") as ppool:

            # ---- constants ----
            w1a = cpool.tile([128, HID], f32, tag="w1a")
            w1b = cpool.tile([128, HID], f32, tag="w1b")
            nc.sync.dma_start(w1a[:], W1_in[0:128, :])
            nc.sync.dma_start(w1b[:], W1_in[128:256, :])
            b1t = cpool.tile([HID, 1], f32, tag="b1t")
            nc.sync.dma_start(b1t[:], b1_in[0:1, :].rearrange("o h -> h o"))
            w2sb = cpool.tile([HID + 1, OUT_SIZE], f32, tag="w2")
            nc.sync.dma_start(w2sb[0:HID, :], W2_in[:])
            nc.sync.dma_start(w2sb[HID:HID + 1, :], b2_in[:])
            betas_sb = cpool.tile([1, 2], f32, tag="betas")
            nc.sync.dma_start(betas_sb[:], betas_in[:])
            ident_f = cpool.tile([128, 128], f32, tag="idf")
            make_identity(nc, ident_f[:])
            ident_b = cpool.tile([128, 128], bf16, tag="idb")
            nc.vector.tensor_copy(ident_b[:], ident_f[:])
            ones1 = cpool.tile([1, 128], f32, tag="ones1")
            nc.gpsimd.memset(ones1[:], 1.0)
            zero_tab = cpool.tile([44, RW], bf16, tag="ztab")
            nc.gpsimd.memset(zero_tab[:], 0.0)
            # zero pad rows of both local tables (disjoint from tile writes below)
            nc.sync.dma_start(tab_in[0][12500:12544, :], zero_tab[:])
            nc.sync.dma_start(tab_in[1][12500:12544, :], zero_tab[:])
            # beta broadcast tiles [128,1] per layer
            beta128 = []
            for l in range(2):
                bp = ppool.tile([128, 1], f32, tag="betap")
                nc.tensor.matmul(bp[:], lhsT=ones1[:], rhs=betas_sb[:, l:l + 1],
                                 start=True, stop=True)
                bl = cpool.tile([128, 1], f32, tag=f"beta{l}")
                nc.vector.tensor_copy(bl[:], bp[:])
                beta128.append(bl)

            # ---- helper: build table row block from h tile ----
            def build_table(h_sb, dst_dram, t, rows):
                # h_sb: [128, HID] f32 (row-major node tile)
                sq = pool.tile([128, HID], f32, tag="sq")
                nc.vector.tensor_mul(sq[:], h_sb[:], h_sb[:])
                n2 = pool.tile([128, 1], f32, tag="n2")
                nc.vector.reduce_sum(n2[:], sq[:], axis=mybir.AxisListType.X)
                # norm = exp(0.5*ln(n2)) ; ln(0) -> -inf -> exp -> 0
                lnn = pool.tile([128, 1], f32, tag="lnn")
                nc.scalar.activation(lnn[:], n2[:], AF.Ln)
                norm = pool.tile([128, 1], f32, tag="norm")
                nc.scalar.activation(norm[:], lnn[:], AF.Exp, scale=0.5)
                nclamp = pool.tile([128, 1], f32, tag="nclamp")
                nc.vector.tensor_scalar_max(nclamp[:], norm[:], EPS)
                rn = pool.tile([128, 1], f32, tag="rn")
                nc.vector.reciprocal(rn[:], nclamp[:])
                tabt = pool.tile([128, RW], bf16, tag="tabt")
                nc.vector.tensor_scalar_mul(tabt[:, 0:HID], h_sb[:], rn[:])
                tf32 = tabt[:].bitcast(f32)          # [128, 64]
                nc.vector.tensor_copy(tf32[:, 16:17], norm[:])
                nc.gpsimd.memset(tf32[:, 17:64], 0.0)
                nc.sync.dma_start(dst_dram[t * 128: t * 128 + rows, :], tabt[:rows, :])

            # ---- lin1: h0 = relu(feats @ W1 + b1), build table 0 ----
            for t in range(TILES):
                xa = pool.tile([128, 128], f32, tag="xa")
                xb = pool.tile([128, 128], f32, tag="xb")
                nc.sync.dma_start(xa[:], featsT[0:128, t * 128:(t + 1) * 128])
                nc.sync.dma_start(xb[:], featsT[128:256, t * 128:(t + 1) * 128])
                hT_p = ppool.tile([HID, 128], f32, tag="hTp")
                nc.tensor.matmul(hT_p[:], lhsT=w1a[:], rhs=xa[:], start=True, stop=False)
                nc.tensor.matmul(hT_p[:], lhsT=w1b[:], rhs=xb[:], start=False, stop=True)
                hT = pool.tile([HID, 128], f32, tag="hT")
                nc.scalar.activation(hT[:], hT_p[:], AF.Relu, bias=b1t[:])
                h_p = ppool.tile([128, HID], f32, tag="hp")
                nc.tensor.transpose(h_p[:], hT[:], ident_f[0:HID, 0:HID])
                h0 = pool.tile([128, HID], f32, tag="h0")
                nc.vector.tensor_copy(h0[:], h_p[:])
                build_table(h0, tab_in[0], t, 84 if t == TILES - 1 else 128)

            # ---- AGNN layers ----
            for l in range(2):
                nc.gpsimd.collective_compute(
                    "AllGather", ALU.bypass,
                    replica_groups=[list(range(NCORES))],
                    ins=[tab_in[l][:]], outs=[tab_out[l][:]],
                )
                for g in range(NGROUPS):
                    slabs = []
                    for b in range(NBANKS):
                        cm = next(c for c in call_meta if c[0] == g and c[1] == b)
                        _, _, coff, n_idx, _ = cm
                        w = n_idx // 16
                        it = pool.tile([128, KB_MAX[b] * 8], mybir.dt.int16, tag=f"idx{b}")
                        nc.sync.dma_start(it[:, :w], idx_in[:, coff:coff + w])
                        sl = spool.tile([128, KB_MAX[b] * RW], bf16, tag=f"slab{b}")
                        KCH = 16  # k-blocks (2048 idx) per gather call
                        ktot = n_idx // 128
                        for kc0 in range(0, ktot, KCH):
                            kcn = min(KCH, ktot - kc0)
                            nn = kcn * 128
                            nc.gpsimd.dma_gather(
                                out_ap=sl[:].rearrange("p (k r) -> p k r", r=RW)[:, kc0:kc0 + kcn, :],
                                in_ap=tab_out[l][b * BROWS:(b + 1) * BROWS, :],
                                idxs_ap=it[:, kc0 * 8: kc0 * 8 + nn // 16],
                                num_idxs=nn, num_idxs_reg=nn, elem_size=RW,
                                single_packet=False,
                            )
                        slabs.append(sl)
                    for ti in range(TG):
                        t = g * TG + ti
                        KS = int(KSUM_T[t])
                        # local k-offsets of this tile's blocks within each bank slab
                        boff = []
                        for b in range(NBANKS):
                            cm = next(c for c in call_meta if c[0] == g and c[1] == b)
                            ks_list = cm[4]
                            boff.append(sum(ks_list[:ti]))
                        # dst tile (xn + beta scale)
                        dn = pool.tile([128, RW], bf16, tag="dn")
                        nc.sync.dma_start(dn[:], tab_in[l][t * 128:(t + 1) * 128, :])
                        dnb = pool.tile([128, HID], bf16, tag="dnb")
                        nc.vector.tensor_scalar_mul(dnb[:], dn[:, 0:HID], beta128[l][:])
                        # M = xn_src * dnb  (per bank block, concat into M)
                        M = pool.tile([128, KS_MAX * HID], bf16, tag="M")
                        ko = 0
                        for b in range(NBANKS):
                            kb = int(KHAT[t, b])
                            if kb == 0:
                                continue
                            sv = slabs[b][:].rearrange("p (k r) -> p k r", r=RW)[
                                :, boff[b]:boff[b] + kb, 0:HID]
                            mv = M[:].rearrange("p (k r) -> p k r", r=HID)[:, ko:ko + kb, :]
                            dnb_b = dnb[:].rearrange("p (a r) -> p a r", a=1).to_broadcast([128, kb, HID])
                            nc.vector.tensor_tensor(mv, sv, dnb_b, op=ALU.mult)
                            ko += kb
                        # dots
                        dots = pool.tile([128, KS_MAX], f32, tag="dots")
                        nc.vector.reduce_sum(
                            dots[:, :KS], M[:].rearrange("p (k r) -> p k r", r=HID)[:, :KS, :],
                            axis=mybir.AxisListType.X)
                        # e = exp(dots), s = sum(e)
                        e = pool.tile([128, KS_MAX], f32, tag="e")
                        s = pool.tile([128, 1], f32, tag="s")
                        nc.scalar.activation(e[:, :KS], dots[:, :KS], AF.Exp,
                                             accum_out=s[:])
                        # denom = max(s - npad, tiny); r = 1/denom
                        npt = pool.tile([128, 1], f32, tag="npt")
                        nc.sync.dma_start(npt[:], npad_in[t * 128:(t + 1) * 128, :])
                        den = pool.tile([128, 1], f32, tag="den")
                        nc.vector.tensor_sub(den[:], s[:], npt[:])
                        den2 = pool.tile([128, 1], f32, tag="den2")
                        nc.vector.tensor_scalar_max(den2[:], den[:], 1e-30)
                        rden = pool.tile([128, 1], f32, tag="rden")
                        nc.vector.reciprocal(rden[:], den2[:])
                        # w = e * norm_src ; M2 = xn_src * w
                        w_t = pool.tile([128, KS_MAX], f32, tag="wt")
                        ko = 0
                        for b in range(NBANKS):
                            kb = int(KHAT[t, b])
                            if kb == 0:
                                continue
                            nv = slabs[b][:].bitcast(f32).rearrange(
                                "p (k r) -> p k r", r=RW // 2)[:, boff[b]:boff[b] + kb, 16:17]
                            nc.vector.tensor_tensor(
                                w_t[:].rearrange("p (k a) -> p k a", a=1)[:, ko:ko + kb, :],
                                e[:].rearrange("p (k a) -> p k a", a=1)[:, ko:ko + kb, :],
                                nv, op=ALU.mult)
                            ko += kb
                        M2 = pool.tile([128, KS_MAX * HID], bf16, tag="M2")
                        ko = 0
                        for b in range(NBANKS):
                            kb = int(KHAT[t, b])
                            if kb == 0:
                                continue
                            sv = slabs[b][:].rearrange("p (k r) -> p k r", r=RW)[
                                :, boff[b]:boff[b] + kb, 0:HID]
                            m2v = M2[:].rearrange("p (k r) -> p k r", r=HID)[:, ko:ko + kb, :]
                            wb = w_t[:].rearrange("p (k a) -> p k a", a=1)[
                                :, ko:ko + kb, :].to_broadcast([128, kb, HID])
                            nc.vector.tensor_tensor(m2v, sv, wb, op=ALU.mult)
                            ko += kb
                        # wsum: accumulate sum_k M2[:,k,:] into psum [128, HID]
                        if PSUM_ACC_TRICK:
                            msg_p = ppool.tile([128, HID], f32, tag="msgp")
                            CH = 16  # k per matmul chunk (16*32 = 512 free)
                            nch = (KS + CH - 1) // CH
                            for ci in range(nch):
                                k0 = ci * CH
                                kc = min(CH, KS - k0)
                                rhs = M2[:].rearrange("p (k r) -> p k r", r=HID)[
                                    :, k0:k0 + kc, :]
                                ob = msg_p[:].rearrange("p (a r) -> p a r", a=1).to_broadcast(
                                    [128, kc, HID])
                                nc.tensor.matmul(ob, lhsT=ident_b[:], rhs=rhs,
                                                 start=(ci == 0), stop=(ci == nch - 1))
                        else:
                            msum = pool.tile([128, HID], f32, tag="msum")
                            nc.vector.reduce_sum(
                                msum[:],
                                M2[:].rearrange("p (k r) -> p r k", r=HID)[:, :, :KS],
                                axis=mybir.AxisListType.X)
                        # h_next = relu(msg * rden)
                        hn = pool.tile([128, HID], f32, tag="hn")
                        src_msg = msg_p[:] if PSUM_ACC_TRICK else msum[:]
                        nc.scalar.activation(hn[:], src_msg, AF.Relu, scale=rden[:])
                        if l == 0:
                            build_table(hn, tab_in[1], t,
                                        84 if t == TILES - 1 else 128)
                        else:
                            # lin2 + log_softmax
                            hT2_p = ppool.tile([HID, 128], f32, tag="hT2p")
                            nc.tensor.transpose(hT2_p[:], hn[:], ident_f[:])
                            hT2 = pool.tile([HID + 1, 128], f32, tag="hT2")
                            nc.vector.tensor_copy(hT2[0:HID, :], hT2_p[:])
                            nc.gpsimd.memset(hT2[HID:HID + 1, :], 1.0)
                            o_p = ppool.tile([128, OUT_SIZE], f32, tag="op")
                            nc.tensor.matmul(o_p[:], lhsT=hT2[:], rhs=w2sb[:],
                                             start=True, stop=True)
                            logits = pool.tile([128, OUT_SIZE], f32, tag="logits")
                            nc.vector.tensor_copy(logits[:], o_p[:])
                            nmax = pool.tile([128, 1], f32, tag="nmax")
                            nc.vector.tensor_reduce(nmax[:], logits[:],
                                                    axis=mybir.AxisListType.X,
                                                    op=ALU.max, negate=True)
                            ex = pool.tile([128, OUT_SIZE], f32, tag="ex")
                            se = pool.tile([128, 1], f32, tag="se")
                            nc.scalar.activation(ex[:], logits[:], AF.Exp,
                                                 bias=nmax[:], accum_out=se[:])
                            lse = pool.tile([128, 1], f32, tag="lse")
                            nc.scalar.activation(lse[:], se[:], AF.Ln)
                            res = pool.tile([128, OUT_SIZE], f32, tag="res")
                            nc.vector.tensor_scalar(
                                res[:], logits[:], scalar1=nmax[:], scalar2=lse[:],
                                op0=ALU.add, op1=ALU.subtract)
                            nc.sync.dma_start(out_t[t * 128:(t + 1) * 128, :], res[:])
    nc.compile()
    return nc


def kernel(edge, features, W1, b1, betas, W2, b2):
    from concourse.bass_utils import run_bass_kernel_spmd
    import ml_dtypes

    edge = np.asarray(edge)
    features = np.asarray(features, dtype=np.float32)
    W1 = np.asarray(W1, dtype=np.float32)
    b1 = np.asarray(b1, dtype=np.float32)
    betas = np.asarray(betas, dtype=np.float32)
    W2 = np.asarray(W2, dtype=np.float32)
    b2 = np.asarray(b2, dtype=np.float32)

    idx_blob, npad, meta = _host_preprocess(edge)
    import hashlib
    key = hashlib.sha256(meta["KHAT"].tobytes()).hexdigest()
    if key not in _cache:
        _cache[key] = _build_program(meta)
    nc = _cache[key]

    core_of = meta["core_of"]; pos_of = meta["pos_of"]
    in_maps = []
    for c in range(NCORES):
        mask = core_of == c
        nodes = np.where(mask)[0]
        posc = pos_of[nodes]
        fT = np.zeros((IN_SIZE, SHARD), dtype=np.float32)
        fT[:, posc] = features[nodes].T
        in_maps.append({
            "featsT": fT,
            "W1": W1, "b1": b1.reshape(1, HID),
            "W2": W2, "b2": b2.reshape(1, OUT_SIZE),
            "betas": betas.reshape(1, 2).astype(np.float32),
            "idx": idx_blob[c],
            "npad": npad[c],
        })
    res = run_bass_kernel_spmd(nc, in_maps, core_ids=list(range(NCORES)))
    out = np.empty((N_NODES, OUT_SIZE), dtype=np.float32)
    for c in range(NCORES):
        oc = res.results[c]["out"]
        mask = core_of == c
        nodes = np.where(mask)[0]
        out[nodes] = oc[pos_of[nodes]]
    return out



# revision 4
# speedup vs baseline: 3.9876x; 3.9876x over previous
"""AGNN (2-layer) distributed Bass kernel for 8 TRN2 NeuronCores.

Design (v2 — wire-lean):
- Nodes degree-sorted and dealt round-robin to 8 cores (12544 padded rows each,
  98 tiles of 128). All index remapping done on host; output un-permuted on host.
- Wire format: features fp8_e4m3 (transposed [256, SHARD] per core), gather
  indices UNreplicated [16, WTOT] int16 (the x8 partition replication the
  dma_gather engine wants is done on-device), output bf16. Total bytes per call
  ~33MB in + ~13MB out vs 157MB + 26MB for the f32/replicated layout.
- Per AGNN layer: each core builds a bf16 table shard (row = [xn 32 | x 32 |
  pad 64] bf16 = 256B), AllGather -> full table in DRAM.
- Messages: dst-major slot grid [128 nodes, K slots] per tile, slots bucketed by
  src bank (4 banks of 25088 rows so dma_gather's int16 indices fit), banks laid
  out contiguously per tile so per-tile compute is single-instruction per stage.
  Bulk row-major gathers via dma_gather (256B rows); pad slots point at a zero
  row.
- Compute: DVE mul + grouped reduce for cos logits, ACT exp (beta folded into
  the activation scale) with fused denominator (accum_out), Sigma-e minus
  host-precomputed pad count, M2 = x_src * e, grouped reduce, ACT relu+scale.
- lin1: node-major fp8 matmul (features tile is the stationary operand, no
  transpose needed), bias via a K=1 ones-row matmul. lin2 + log_softmax fused
  per tile, emitted bf16.
"""

import numpy as np

N_NODES = 100000
N_EDGES = 1600000
IN_SIZE = 256
HID = 32
OUT_SIZE = 64
EPS = 1e-12

NCORES = 8
TILES = 98
SHARD = TILES * 128            # 12544
PAD_NODES = NCORES * SHARD     # 100352
NBANKS = 4
BROWS = 2 * SHARD              # 25088 rows per bank (2 shards)
RW = 128                       # bf16 elems per table row = 256B
DUMMY_LOCAL = 12500            # zero row within the first shard of each bank
KCH = 16                       # k-blocks (2048 idx) per gather call

_cache = {}


def _host_preprocess(edge):
    src = np.asarray(edge[0], dtype=np.int64)
    dst = np.asarray(edge[1], dtype=np.int64)
    deg = np.bincount(dst, minlength=N_NODES)
    order = np.argsort(-deg, kind="stable")      # node ids, heavy first
    rank = np.empty(N_NODES, dtype=np.int64)
    rank[order] = np.arange(N_NODES)
    core_of = rank % NCORES
    pos_of = rank // NCORES                      # 0..12499
    grow_of = core_of * SHARD + pos_of           # global padded table row

    # Pass 2: re-sort nodes WITHIN each shard by per-bank src-count vector.
    # Within-shard reordering never changes any node's bank (banks = 2 whole
    # shards), so bank counts computed from the pass-1 layout stay valid.
    bank1 = grow_of[src] // BROWS
    cnt = np.zeros((N_NODES, NBANKS), dtype=np.int32)
    np.add.at(cnt, (dst, bank1), 1)
    for c in range(NCORES):
        nodes_c = np.where(core_of == c)[0]
        key = np.lexsort((-cnt[nodes_c, 3], -cnt[nodes_c, 2],
                          -cnt[nodes_c, 1], -cnt[nodes_c, 0]))
        pos_of[nodes_c[key]] = np.arange(len(nodes_c))
    grow_of = core_of * SHARD + pos_of

    e_core = core_of[dst]
    e_tile = pos_of[dst] // 128
    e_p = pos_of[dst] % 128
    e_srow = grow_of[src]
    e_bank = e_srow // BROWS
    e_local = e_srow % BROWS

    # counts per (core, tile, p, bank)
    key = ((e_core * TILES + e_tile) * 128 + e_p) * NBANKS + e_bank
    counts = np.bincount(key, minlength=NCORES * TILES * 128 * NBANKS)
    counts = counts.reshape(NCORES, TILES, 128, NBANKS)
    KHAT = counts.max(axis=(0, 2))               # [TILES, NBANKS]
    # k-rank of each edge within its (core,tile,p,bank) cell
    sort_idx = np.argsort(key, kind="stable")
    ks = key[sort_idx]
    first = np.r_[True, ks[1:] != ks[:-1]]
    grp_start = np.maximum.accumulate(np.where(first, np.arange(len(ks)), 0))
    e_k = np.empty(len(ks), dtype=np.int64)
    e_k[sort_idx] = np.arange(len(ks)) - grp_start

    # slot grids: per (core, tile, bank): [KHAT[t,b], 128] int16 local idx
    koff = np.zeros((TILES, NBANKS), dtype=np.int64)   # k-offset of (t,b) within tile's concat
    run = np.cumsum(KHAT, axis=1)
    koff[:, 1:] = run[:, :-1]
    KSUM_T = KHAT.sum(axis=1)                          # slots-k per tile
    tile_off = np.r_[0, np.cumsum(KSUM_T)][:-1]        # k-offset of tile within core stream
    TOTK = int(KSUM_T.sum())

    grid = np.full((NCORES, TOTK, 128), DUMMY_LOCAL, dtype=np.int16)
    flat_k = tile_off[e_tile] + koff[e_tile, e_bank] + e_k
    grid[e_core, flat_k, e_p] = e_local.astype(np.int16)

    # per-(tile,bank) gather streams: slot j = k*128+p order, wrapped in 16
    # partitions (idx j -> [j%16, j//16]); NOT replicated (done on device)
    blobs = []
    call_meta = {}   # (t, b) -> (col offset in blob, n_idx)
    col_off = 0
    for t in range(TILES):
        for b in range(NBANKS):
            kb = int(KHAT[t, b])
            if kb == 0:
                call_meta[(t, b)] = (col_off, 0)
                continue
            st = grid[:, tile_off[t] + koff[t, b]: tile_off[t] + koff[t, b] + kb, :]
            stream = st.reshape(NCORES, -1)             # [NCORES, kb*128]
            w = kb * 128 // 16
            wrapped = stream.reshape(NCORES, w, 16).transpose(0, 2, 1)  # [NCORES,16,w]
            blobs.append(wrapped)
            call_meta[(t, b)] = (col_off, kb * 128)
            col_off += w
    idx_blob = np.ascontiguousarray(np.concatenate(blobs, axis=2))  # [NCORES, 16, WTOT]

    npad = (np.broadcast_to(KSUM_T[None, :, None], (NCORES, TILES, 128))
            - counts.sum(axis=3))                       # [NCORES, TILES, 128]
    npad = np.ascontiguousarray(
        npad.transpose(0, 2, 1)).astype(np.float32)     # [NCORES, 128, TILES]

    meta = {
        "KHAT": KHAT, "KSUM_T": KSUM_T, "call_meta": call_meta,
        "WTOT": int(idx_blob.shape[2]),
        "order": order, "core_of": core_of, "pos_of": pos_of,
    }
    return idx_blob, npad, meta


def _build_program(meta):
    import concourse.bass as bass
    import concourse.bacc as bacc
    import concourse.mybir as mybir
    import concourse.tile as tile
    from concourse.masks import make_identity

    f32 = mybir.dt.float32
    bf16 = mybir.dt.bfloat16
    f8 = mybir.dt.float8e4
    AF = mybir.ActivationFunctionType
    ALU = mybir.AluOpType

    KHAT = meta["KHAT"]; call_meta = meta["call_meta"]; WTOT = meta["WTOT"]
    KSUM_T = meta["KSUM_T"]
    KS_MAX = int(max(KSUM_T))

    nc = bacc.Bacc("TRN2", target_bir_lowering=False, debug=False,
                   enable_asserts=False, num_devices=NCORES)
    featsT = nc.dram_tensor("featsT", [IN_SIZE, SHARD], f8, kind="ExternalInput")
    W1_in = nc.dram_tensor("W1", [IN_SIZE, HID], f8, kind="ExternalInput")
    b1_in = nc.dram_tensor("b1", [1, HID], f32, kind="ExternalInput")
    W2_in = nc.dram_tensor("W2", [HID, OUT_SIZE], f32, kind="ExternalInput")
    b2_in = nc.dram_tensor("b2", [1, OUT_SIZE], f32, kind="ExternalInput")
    betas_in = nc.dram_tensor("betas", [1, 2], f32, kind="ExternalInput")
    idx_in = nc.dram_tensor("idx", [16, WTOT], mybir.dt.int16, kind="ExternalInput")
    npad_in = nc.dram_tensor("npad", [128, TILES], f32, kind="ExternalInput")
    out_t = nc.dram_tensor("out", [SHARD, OUT_SIZE], bf16, kind="ExternalOutput")

    tab_in = [nc.dram_tensor(f"tabin{l}", [SHARD, RW], bf16, kind="Internal")
              for l in range(2)]
    tab_out = [nc.dram_tensor(f"tabout{l}", [PAD_NODES, RW], bf16, kind="Internal",
                              addr_space="Shared") for l in range(2)]

    with tile.TileContext(nc) as tc:
        with tc.tile_pool(name="const", bufs=1) as cpool, \
             tc.tile_pool(name="work", bufs=2) as pool, \
             tc.tile_pool(name="slab", bufs=2) as spool, \
             tc.tile_pool(name="psum", bufs=2, space="PSUM") as ppool:

            # ---- constants / resident tiles ----
            w1a = cpool.tile([128, HID], f8, tag="w1a")
            w1b = cpool.tile([128, HID], f8, tag="w1b")
            nc.sync.dma_start(w1a[:], W1_in[0:128, :])
            nc.sync.dma_start(w1b[:], W1_in[128:256, :])
            b1sb = cpool.tile([1, HID], f32, tag="b1")
            nc.sync.dma_start(b1sb[:], b1_in[:])
            w2sb = cpool.tile([HID + 1, OUT_SIZE], f32, tag="w2")
            nc.sync.dma_start(w2sb[0:HID, :], W2_in[:])
            nc.sync.dma_start(w2sb[HID:HID + 1, :], b2_in[:])
            betas_sb = cpool.tile([1, 2], f32, tag="betas")
            nc.sync.dma_start(betas_sb[:], betas_in[:])
            ident_f = cpool.tile([128, 128], f32, tag="idf")
            make_identity(nc, ident_f[:])
            ones1 = cpool.tile([1, 128], f32, tag="ones1")
            nc.gpsimd.memset(ones1[:], 1.0)
            zero_tab = cpool.tile([44, RW], bf16, tag="ztab")
            nc.gpsimd.memset(zero_tab[:], 0.0)
            # zero pad rows of both local tables (disjoint from tile writes)
            nc.sync.dma_start(tab_in[0][12500:12544, :], zero_tab[:])
            nc.sync.dma_start(tab_in[1][12500:12544, :], zero_tab[:])
            # gather indices: [16, WTOT] from DRAM, replicated x8 on device
            idx_sb = cpool.tile([128, WTOT], mybir.dt.int16, tag="idx")
            for r in range(8):
                nc.sync.dma_start(idx_sb[16 * r:16 * r + 16, :], idx_in[:])
            # pad counts, one column per tile
            npad_sb = cpool.tile([128, TILES], f32, tag="npad")
            nc.sync.dma_start(npad_sb[:], npad_in[:])
            # beta broadcast tiles [128,1] per layer
            beta128 = []
            for l in range(2):
                bp = ppool.tile([128, 1], f32, tag="betap")
                nc.tensor.matmul(bp[:], lhsT=ones1[:], rhs=betas_sb[:, l:l + 1],
                                 start=True, stop=True)
                bl = cpool.tile([128, 1], f32, tag=f"beta{l}")
                nc.vector.tensor_copy(bl[:], bp[:])
                beta128.append(bl)
            # resident table shards (this core's rows): [xn 32 | x 32 | pad]
            tabs = [cpool.tile([128, TILES * RW], bf16, tag=f"tab{l}",
                               name=f"tab{l}")
                    for l in range(2)]

            # ---- helper: build table row block from h tile ----
            def build_table(h_sb, l, t):
                # h_sb: [128, HID] f32 (row-major node tile)
                seg = tabs[l][:, t * RW:(t + 1) * RW]
                sq = pool.tile([128, HID], f32, tag="sq")
                nc.vector.tensor_mul(sq[:], h_sb[:], h_sb[:])
                n2 = pool.tile([128, 1], f32, tag="n2")
                nc.vector.reduce_sum(n2[:], sq[:], axis=mybir.AxisListType.X)
                n2c = pool.tile([128, 1], f32, tag="n2c")
                nc.vector.tensor_scalar_max(n2c[:], n2[:], EPS * EPS)
                nrm = pool.tile([128, 1], f32, tag="nrm")
                nc.scalar.activation(nrm[:], n2c[:], AF.Sqrt)
                rn = pool.tile([128, 1], f32, tag="rn")
                nc.vector.reciprocal(rn[:], nrm[:])
                nc.vector.tensor_scalar_mul(seg[:, 0:HID], h_sb[:], rn[:])
                nc.vector.tensor_copy(seg[:, HID:2 * HID], h_sb[:])
                rows = 84 if t == TILES - 1 else 128
                nc.sync.dma_start(tab_in[l][t * 128: t * 128 + rows, :],
                                  seg[:rows, :])

            # ---- lin1: h0 = relu(feats @ W1 + b1), build table 0 ----
            for t in range(TILES):
                xa = pool.tile([128, 128], f8, tag="xa")
                xb = pool.tile([128, 128], f8, tag="xb")
                nc.sync.dma_start(xa[:], featsT[0:128, t * 128:(t + 1) * 128])
                nc.sync.dma_start(xb[:], featsT[128:256, t * 128:(t + 1) * 128])
                h_p = ppool.tile([128, HID], f32, tag="hp")
                nc.tensor.matmul(h_p[:], lhsT=xa[:], rhs=w1a[:], start=True, stop=False)
                nc.tensor.matmul(h_p[:], lhsT=xb[:], rhs=w1b[:], start=False, stop=False)
                nc.tensor.matmul(h_p[:], lhsT=ones1[:], rhs=b1sb[:], start=False, stop=True)
                h0 = pool.tile([128, HID], f32, tag="h0")
                nc.scalar.activation(h0[:], h_p[:], AF.Relu)
                build_table(h0, 0, t)

            # ---- AGNN layers ----
            for l in range(2):
                nc.gpsimd.collective_compute(
                    "AllGather", ALU.bypass,
                    replica_groups=[list(range(NCORES))],
                    ins=[tab_in[l][:]], outs=[tab_out[l][:]],
                )
                for t in range(TILES):
                    KS = int(KSUM_T[t])
                    # gather all 4 banks into one per-tile slab (k-contiguous)
                    sl = spool.tile([128, KS_MAX * RW], bf16, tag="slab")
                    slv = sl[:].rearrange("p (k r) -> p k r", r=RW)
                    ko = 0
                    for b in range(NBANKS):
                        kb = int(KHAT[t, b])
                        if kb == 0:
                            continue
                        coff, _ = call_meta[(t, b)]
                        for kc0 in range(0, kb, KCH):
                            kcn = min(KCH, kb - kc0)
                            nn = kcn * 128
                            nc.gpsimd.dma_gather(
                                out_ap=slv[:, ko + kc0: ko + kc0 + kcn, :],
                                in_ap=tab_out[l][b * BROWS:(b + 1) * BROWS, :],
                                idxs_ap=idx_sb[:, coff + kc0 * 8:
                                               coff + kc0 * 8 + nn // 16],
                                num_idxs=nn, num_idxs_reg=nn, elem_size=RW,
                                single_packet=False,
                            )
                        ko += kb
                    # cos logits: M = xn_src * xn_dst ; dots = sum_r M
                    xnd = tabs[l][:, t * RW: t * RW + HID]
                    xnd_b = xnd.rearrange("p (a r) -> p a r", a=1).to_broadcast(
                        [128, KS, HID])
                    M = pool.tile([128, KS_MAX * HID], bf16, tag="M")
                    Mv = M[:].rearrange("p (k r) -> p k r", r=HID)
                    nc.vector.tensor_tensor(Mv[:, :KS, :], slv[:, :KS, 0:HID],
                                            xnd_b, op=ALU.mult)
                    dots = pool.tile([128, KS_MAX], f32, tag="dots")
                    nc.vector.reduce_sum(dots[:, :KS], Mv[:, :KS, :],
                                         axis=mybir.AxisListType.X)
                    # e = exp(beta*dots), s = sum(e); denom minus pad count
                    e = pool.tile([128, KS_MAX], f32, tag="e")
                    s = pool.tile([128, 1], f32, tag="s")
                    nc.scalar.activation(e[:, :KS], dots[:, :KS], AF.Exp,
                                         scale=beta128[l][:], accum_out=s[:])
                    den = pool.tile([128, 1], f32, tag="den")
                    nc.vector.tensor_scalar(den[:], s[:],
                                            scalar1=npad_sb[:, t:t + 1],
                                            scalar2=1e-30,
                                            op0=ALU.subtract, op1=ALU.max)
                    rden = pool.tile([128, 1], f32, tag="rden")
                    nc.vector.reciprocal(rden[:], den[:])
                    # M2 = x_src * e ; msum = sum_k M2
                    e_b = e[:].rearrange("p (k a) -> p k a", a=1)[
                        :, :KS, :].to_broadcast([128, KS, HID])
                    M2 = pool.tile([128, KS_MAX * HID], bf16, tag="M2")
                    M2v = M2[:].rearrange("p (k r) -> p k r", r=HID)
                    nc.vector.tensor_tensor(M2v[:, :KS, :],
                                            slv[:, :KS, HID:2 * HID],
                                            e_b, op=ALU.mult)
                    msum = pool.tile([128, HID], f32, tag="msum")
                    nc.vector.reduce_sum(
                        msum[:],
                        M2[:].rearrange("p (k r) -> p r k", r=HID)[:, :, :KS],
                        axis=mybir.AxisListType.X)
                    # h_next = relu(msum * rden)
                    hn = pool.tile([128, HID], f32, tag="hn")
                    nc.scalar.activation(hn[:], msum[:], AF.Relu, scale=rden[:])
                    if l == 0:
                        build_table(hn, 1, t)
                    else:
                        # lin2 + log_softmax
                        hT2_p = ppool.tile([HID, 128], f32, tag="hT2p")
                        nc.tensor.transpose(hT2_p[:], hn[:], ident_f[:])
                        hT2 = pool.tile([HID + 1, 128], f32, tag="hT2")
                        nc.vector.tensor_copy(hT2[0:HID, :], hT2_p[:])
                        nc.gpsimd.memset(hT2[HID:HID + 1, :], 1.0)
                        o_p = ppool.tile([128, OUT_SIZE], f32, tag="op")
                        nc.tensor.matmul(o_p[:], lhsT=hT2[:], rhs=w2sb[:],
                                         start=True, stop=True)
                        nmax = pool.tile([128, 1], f32, tag="nmax")
                        nc.vector.tensor_reduce(nmax[:], o_p[:],
                                                axis=mybir.AxisListType.X,
                                                op=ALU.max, negate=True)
                        ex = pool.tile([128, OUT_SIZE], f32, tag="ex")
                        se = pool.tile([128, 1], f32, tag="se")
                        nc.scalar.activation(ex[:], o_p[:], AF.Exp,
                                             bias=nmax[:], accum_out=se[:])
                        lse = pool.tile([128, 1], f32, tag="lse")
                        nc.scalar.activation(lse[:], se[:], AF.Ln)
                        res = pool.tile([128, OUT_SIZE], bf16, tag="res")
                        nc.vector.tensor_scalar(
                            res[:], o_p[:], scalar1=nmax[:], scalar2=lse[:],
                            op0=ALU.add, op1=ALU.subtract)
                        nc.sync.dma_start(out_t[t * 128:(t + 1) * 128, :], res[:])
    nc.compile()
    return nc


def _make_in_maps(features, W1, b1, betas, W2, b2, idx_blob, npad, meta):
    import ml_dtypes

    core_of = meta["core_of"]; pos_of = meta["pos_of"]
    W1_q = np.asarray(W1, np.float32).astype(ml_dtypes.float8_e4m3)
    b1_r = np.asarray(b1, np.float32).reshape(1, HID)
    W2_r = np.asarray(W2, np.float32)
    b2_r = np.asarray(b2, np.float32).reshape(1, OUT_SIZE)
    betas_r = np.asarray(betas, np.float32).reshape(1, 2)
    feats_q = np.asarray(features, np.float32).astype(ml_dtypes.float8_e4m3)
    in_maps = []
    for c in range(NCORES):
        nodes = np.where(core_of == c)[0]
        posc = pos_of[nodes]
        fT = np.zeros((IN_SIZE, SHARD), dtype=ml_dtypes.float8_e4m3)
        fT[:, posc] = feats_q[nodes].T
        in_maps.append({
            "featsT": fT,
            "W1": W1_q, "b1": b1_r,
            "W2": W2_r, "b2": b2_r,
            "betas": betas_r,
            "idx": idx_blob[c],
            "npad": npad[c],
        })
    return in_maps


def kernel(edge, features, W1, b1, betas, W2, b2):
    from concourse.bass_utils import run_bass_kernel_spmd

    edge = np.asarray(edge)
    idx_blob, npad, meta = _host_preprocess(edge)
    import hashlib
    key = hashlib.sha256(meta["KHAT"].tobytes()).hexdigest()
    if key not in _cache:
        _cache[key] = _build_program(meta)
    nc = _cache[key]

    in_maps = _make_in_maps(features, W1, b1, betas, W2, b2,
                            idx_blob, npad, meta)
    res = run_bass_kernel_spmd(nc, in_maps, core_ids=list(range(NCORES)))
    core_of = meta["core_of"]; pos_of = meta["pos_of"]
    out = np.empty((N_NODES, OUT_SIZE), dtype=np.float32)
    for c in range(NCORES):
        oc = np.asarray(res.results[c]["out"], dtype=np.float32)
        nodes = np.where(core_of == c)[0]
        out[nodes] = oc[pos_of[nodes]]
    return out
